# revision 23
# baseline (speedup 1.0000x reference)
"""AdaPT int8-quantized 3x3 conv (exact) on 8 TRN2 NeuronCores — v3.

Full inputs: x [32,8,384,384] f32, weight [8,8,3,3] f32, bias [8] f32.

Sharding: batch x height grid (2 batch-halves x 4 row-strips of 96 rows).
Each core gets x in a ROW-SLOT layout prepared on the host:
  x_core [100, CI=8, IMG=16, 384] f32, where row r is strip row r-1
  (rows 0 and 97..99 are zero halo/pad).

Per-core plan (partition p = 8*v + ci, v = row slot 0..15):
  - pass 1: load f32 pieces (SP/ACT/Pool queues), abs-max them (DVE XYZ /
    Pool XYZWC), discard.  AllGather + local max -> global amax -> sx.
  - weight prep on a single partition: wrow [1,576] -> quantize exactly
    (f32 magic round) -> DRAM-transpose -> wkt -> qw_s [24,24] -> 14 DMAs
    build w3[kx][(8v+ci), (8p+co)] = qw[co,ci,v-p] in bf16.
  - pass 2: reload pieces; quantize in two robust steps:
    op1 (in place, f32): t = x*sx + 12582912  (f32 add rounds RNE at the
    2^23 integer boundary -> t = 2^23*1.5 + round(x*sx) exactly);
    op2: xq = bf16(t - 12582912)  (exact integer in [-127,127] -> the
    bf16 cast is exact regardless of the hardware rounding mode).
    Signed bf16 inputs keep |psum| <= 1.16e6 (exact in fp32, and exact
    even if the PE datapath is bf16-native).
  - conv: flat column stream of C = 16*7*386 cols/partition; per 512-col
    psum chunk 3 accumulating matmuls (kx = 0,1,2; rhs offset kx-1).
    Each column yields 14 output rows x 8 co (p = 0..13, m = 8p+co).
  - evac psum[112, n]*inv + bias on ACT/DVE into bf16 staging, contiguous
    DMA to out [112, C] bf16 (permuted layout); host upcasts/unpermutes.
"""

import numpy as np

N_CORES = 8
IMG = 16         # images per core
CI = 8
CO = 8
H = W = 384
WP = W + 2       # padded row width: col 0 and 385 are pads
ROWS = 96        # output rows per core strip
NK = 7           # row groups per strip
P = 14           # output rows per group (last group: 12 real)
V = 16           # row slots (P + 2 halo)
XROWS = 100      # x_core rows: strip rows -1..98 (97..98 zero)
NM = CO * P      # 112 psum partitions (m = 8*p + co)
C = IMG * NK * WP                 # 43232 flat stream cols per partition
MAX_Q = 127.0
WMAGIC = 12582912.0  # f32 round-to-int offset (1.5 * 2**23)
CHUNK = 512
NCHUNK = -(-(C - 2) // CHUNK)     # 85: sweep cols [1, C-1)
OGRP = 4         # chunks per out DMA group
NS = 6           # stage buffers

_cached = {}


def _build(n_cores=N_CORES, debug=False):
    from concourse import bacc, bass, tile, mybir, bass_isa

    f32 = mybir.dt.float32
    bf16 = mybir.dt.bfloat16

    nc = bacc.Bacc(
        "TRN2", target_bir_lowering=False, debug=debug, num_devices=n_cores
    )

    x_ext = nc.declare_dram_parameter("x", [XROWS, CI, IMG, W], f32, isOutput=False)
    w_ext = nc.declare_dram_parameter("weight", [CO, CI, 3, 3], f32, isOutput=False)
    b_ext = nc.declare_dram_parameter("bias", [CO], f32, isOutput=False)
    out_ext = nc.declare_dram_parameter("out", [NM, C], bf16, isOutput=True)

    sb = lambda name, shape, dt: nc.alloc_sbuf_tensor(name, list(shape), dt).ap()
    xq = sb("xq_s", [128, IMG, NK, WP], bf16)    # quantized x (signed)
    stage = [sb(f"stg{i}_s", [128, 8, 1, W], f32) for i in range(NS)]
    wrow = sb("wrow_s", [1, CO * CI * 9], f32)   # (co,ci,ky,kx) flat
    qrow = sb("qrow_s", [1, CO * CI * 9], bf16)
    qrowt = sb("qrowt_s", [1, CO * CI * 9], bf16)  # (ci,ky,kx,co) flat
    wkt = sb("wkt_s", [CI, 9 * CO], bf16)        # [ci, (ky,kx,co)]
    qw_s = sb("qws_s", [24, 3 * CO], bf16)       # [8ky+ci, 8kx+co]
    w3 = sb("w3_s", [128, 3, NM], bf16)          # lhsT per kx tap
    brow = sb("brow_s", [1, CO], f32)            # bias row
    aw0 = sb("aw0_s", [1, 1], f32)
    sw0 = sb("sw0_s", [1, 1], f32)
    inv0 = sb("inv0_s", [1, 1], f32)
    bias_e = sb("biase_s", [NM, 1], f32)         # bias[co] at partition m
    axd = [sb(f"axd{j}_s", [128, 1], f32) for j in range(8)]   # DVE partials
    axp = [sb(f"axp{j}_s", [1, 1], f32) for j in range(8)]     # Pool partials
    ax_all = sb("axall_s", [128, 1], f32)
    ax0 = sb("ax0_s", [1, 1], f32)
    axg8 = sb("axg8_s", [1, n_cores], f32)
    axg0 = sb("axg0_s", [1, 1], f32)
    axg = sb("axg_s", [128, 1], f32)
    sx = sb("sx_s", [128, 1], f32)
    inv = sb("inv_s", [NM, 1], f32)

    xq_f = xq.rearrange("p a b c -> p (a b c)")  # [128, C] flat stream
    w3_f = w3.rearrange("p a b -> p (a b)")

    with tile.TileContext(nc) as tc:
        with (
            tc.tile_pool(name="st", bufs=3) as spool,
            tc.tile_pool(name="psum", bufs=8, space="PSUM") as pspool,
            tc.tile_pool(name="dram", bufs=1, space="DRAM") as dpool,
        ):
            SP, ACT, DVE, POOL = nc.sync, nc.scalar, nc.vector, nc.gpsimd

            # pads of the quantized stream are zero
            nc.vector.memset(xq[:, :, :, 0:1], 0.0)
            nc.vector.memset(xq[:, :, :, WP - 1:WP], 0.0)
            # one-time tiny loads first (0.5us, ahead of the big x loads)
            ACT.dma_start(out=wrow[:, :], in_=w_ext[:, :, :, :])
            SP.dma_start(out=brow[:, :], in_=b_ext[:])

            # ---------------- pass 1: load + amax, then discard --------------
            # loads: Pool first 2 (its amax work comes later), then SP/ACT;
            # amax: DVE takes early pieces, Pool late ones; tail split.
            load_as = [POOL, POOL, SP, ACT, SP, ACT, SP, ACT, SP, ACT, SP,
                       ACT, SP, ACT]
            amax_as = [DVE, DVE, DVE, DVE, POOL, DVE, POOL, DVE, POOL, POOL,
                       POOL, POOL, DVE, POOL]
            nd = np_ = 0
            for pc in range(14):
                k, hh = divmod(pc, 2)
                stg = stage[pc % NS]
                load_as[pc].dma_start(
                    out=stg[:, :, :, :],
                    in_=x_ext[14 * k:14 * k + 16, :, 8 * hh:8 * hh + 8, :],
                )
                if amax_as[pc] is DVE:
                    DVE.tensor_reduce(
                        axd[nd][:, :], stg[:, :, :, :],
                        mybir.AxisListType.XYZ, mybir.AluOpType.max,
                        apply_absolute_value=True,
                    )
                    nd += 1
                else:
                    POOL.tensor_reduce(
                        axp[np_][:, :], stg[:, :, :, :],
                        mybir.AxisListType.XYZWC, mybir.AluOpType.max,
                        apply_absolute_value=True,
                    )
                    np_ += 1

            # ---------------- weight prep (DVE ops + SP/ACT DMAs) ------------
            nc.vector.tensor_reduce(
                aw0[:, :], wrow[:, :], mybir.AxisListType.X,
                mybir.AluOpType.max, apply_absolute_value=True,
            )
            nc.vector.reciprocal(sw0[:, :], aw0[:, :])
            nc.vector.tensor_scalar(
                out=sw0[:, :], in0=sw0[:, :], scalar1=MAX_Q, scalar2=None,
                op0=mybir.AluOpType.mult,
            )
            # exact integer quantization of weights (f32 magic round)
            nc.vector.tensor_scalar(
                out=wrow[:, :], in0=wrow[:, :], scalar1=sw0[:, :],
                scalar2=WMAGIC,
                op0=mybir.AluOpType.mult, op1=mybir.AluOpType.add,
            )
            nc.vector.tensor_scalar(
                out=qrow[:, :], in0=wrow[:, :], scalar1=-WMAGIC, scalar2=None,
                op0=mybir.AluOpType.add,
            )
            with nc.allow_non_contiguous_dma(reason="one-time w scatter"):
                # transpose via DRAM: qrow (co,(ci,ky,kx)) -> qrowt ((..),co)
                wscr = dpool.tile([1, CO * CI * 9], bf16)
                SP.dma_start(out=wscr[:, :], in_=qrow[:, :])
                ACT.dma_start(
                    out=qrowt[:, :],
                    in_=wscr.rearrange("p (co r) -> p r co", co=CO),
                )
                # wkt[ci, (ky,kx,co)]: partition-expand, both contiguous
                SP.dma_start(out=wkt[:, :], in_=qrowt[:, :])
                # qw_s[8ky+ci, (kx,co)] <- wkt[ci, ky-block]
                for ky in range(3):
                    ACT.dma_start(
                        out=qw_s[8 * ky:8 * ky + 8, :],
                        in_=wkt[:, 24 * ky:24 * ky + 24],
                    )
                # w3[kx][8(p+ky)+ci, 8p+co]: one DMA per p
                nc.vector.memset(w3[:, :, :], 0.0)
                for p in range(P):
                    SP.dma_start(
                        out=w3[8 * p:8 * p + 24, :, 8 * p:8 * p + 8],
                        in_=qw_s[:, :],
                    )
            # bias scatter to evac partitions (m = 8p+co) via doubling
            SP.dma_start(out=bias_e[0:CO, :], in_=brow[:, :])
            for m0 in (8, 16, 32, 64):
                m1 = min(2 * m0, NM)
                SP.dma_start(out=bias_e[m0:m1, :], in_=bias_e[0:m1 - m0, :])

            # ---------------- combine amax partials ----------------
            for j in range(1, nd):
                nc.vector.tensor_tensor(
                    out=axd[0][:, :], in0=axd[0][:, :], in1=axd[j][:, :],
                    op=mybir.AluOpType.max,
                )
            for j in range(1, np_):
                nc.vector.tensor_tensor(
                    out=axp[0][:, :], in0=axp[0][:, :], in1=axp[j][:, :],
                    op=mybir.AluOpType.max,
                )
            nc.gpsimd.partition_all_reduce(
                ax_all[:, :], axd[0][:, :], channels=128,
                reduce_op=bass_isa.ReduceOp.max,
            )
            nc.vector.tensor_tensor(
                out=ax0[:, :], in0=ax_all[0:1, :], in1=axp[0][:, :],
                op=mybir.AluOpType.max,
            )
            # ------------- amax exchange: AllGather + local max -------------
            cc_in = dpool.tile([1, 1], f32)
            cc_out = dpool.tile([1, n_cores], f32)
            POOL.dma_start(out=cc_in[:, :], in_=ax0[:, :])
            nc.gpsimd.collective_compute(
                "AllGather",
                mybir.AluOpType.bypass,
                replica_groups=[list(range(n_cores))],
                ins=[cc_in.opt()],
                outs=[cc_out.opt()],
            )
            POOL.dma_start(out=axg8[:, :], in_=cc_out[:, :])
            nc.vector.tensor_reduce(
                axg0[:, :], axg8[:, :], mybir.AxisListType.X,
                mybir.AluOpType.max,
            )
            nc.gpsimd.partition_broadcast(axg[:, :], axg0[:, :])
            # sx = 127/axg (per-partition, all equal)
            nc.vector.reciprocal(sx[:, :], axg[:, :])
            nc.vector.tensor_scalar(
                out=sx[:, :], in0=sx[:, :], scalar1=MAX_Q, scalar2=None,
                op0=mybir.AluOpType.mult,
            )
            # inv = axg * aw / 127^2 at evac partitions
            nc.vector.tensor_tensor(
                out=inv0[:, :], in0=axg0[:, :], in1=aw0[:, :],
                op=mybir.AluOpType.mult,
            )
            nc.vector.tensor_scalar(
                out=inv0[:, :], in0=inv0[:, :], scalar1=1.0 / (MAX_Q * MAX_Q),
                scalar2=None, op0=mybir.AluOpType.mult,
            )
            nc.gpsimd.partition_broadcast(inv[:, :], inv0[:, :])

            # ------------- pass 2: reload + 2-step quantize (img-major) ------
            # pieces: (hh, k) img-major; first two k of hh=0 split in half
            # so the conv can start as soon as possible.
            pieces = []
            for hh in range(2):
                for k in range(NK):
                    if hh == 0 and k < 2:
                        pieces.append((k, 0, 4))
                        pieces.append((k, 4, 4))
                    else:
                        pieces.append((k, 8 * hh, 8))
            load2_as = [SP, ACT, SP, ACT, POOL, SP, ACT, SP, ACT, SP, ACT,
                        POOL, SP, ACT, SP, ACT]
            op1_as = [DVE, POOL, DVE, POOL, DVE, POOL, DVE, POOL, DVE, POOL,
                      DVE, POOL, DVE, POOL, DVE, POOL]
            op2_as = [POOL, DVE, POOL, DVE, ACT, ACT, ACT, ACT, ACT, ACT,
                      ACT, ACT, ACT, ACT, ACT, ACT]
            for pc2, (k, i0, ni) in enumerate(pieces):
                stg = stage[(14 + pc2) % NS]
                load2_as[pc2].dma_start(
                    out=stg[:, 0:ni, :, :],
                    in_=x_ext[14 * k:14 * k + 16, :, i0:i0 + ni, :],
                )
                # op1: t = x*sx + WMAGIC (in place, f32, RNE at 2^23)
                op1_as[pc2].tensor_scalar(
                    out=stg[:, 0:ni, :, :], in0=stg[:, 0:ni, :, :],
                    scalar1=sx[:, :], scalar2=WMAGIC,
                    op0=mybir.AluOpType.mult, op1=mybir.AluOpType.add,
                )
                # op2: xq = bf16(t - WMAGIC) (exact integer)
                if op2_as[pc2] is ACT:
                    ACT.activation(
                        xq[:, i0:i0 + ni, k:k + 1, 1:W + 1],
                        stg[:, 0:ni, :, :],
                        mybir.ActivationFunctionType.Copy,
                        bias=-WMAGIC, scale=1.0,
                    )
                else:
                    op2_as[pc2].tensor_scalar(
                        out=xq[:, i0:i0 + ni, k:k + 1, 1:W + 1],
                        in0=stg[:, 0:ni, :, :],
                        scalar1=-WMAGIC, scalar2=None,
                        op0=mybir.AluOpType.add,
                    )

            # ---------------- conv: 3 taps x 85 chunks ----------------
            evac_cycle = (nc.scalar, nc.vector)
            out_q = (SP, POOL)
            ch = 0
            g_out = 0
            while ch < NCHUNK:
                nb = min(OGRP, NCHUNK - ch)
                if NCHUNK - ch - nb == 1:
                    nb -= 1          # keep a 1-chunk final group (short tail)
                j0 = 1 + CHUNK * ch
                glen = min(C - 1, j0 + CHUNK * nb) - j0
                st = spool.tile([NM, OGRP * CHUNK], bf16, tag="st")
                for b in range(nb):
                    ja = 1 + CHUNK * (ch + b)
                    N = min(CHUNK, C - 1 - ja)
                    ps = pspool.tile([NM, CHUNK], f32, tag="ps")
                    for kx in range(3):
                        nc.tensor.matmul(
                            ps[:, 0:N],
                            w3_f[:, NM * kx:NM * kx + NM],
                            xq_f[:, ja + kx - 1:ja + kx - 1 + N],
                            start=(kx == 0),
                            stop=(kx == 2),
                        )
                    eng = evac_cycle[(ch + b) % 2]
                    if eng is nc.scalar:
                        nc.scalar.activation(
                            st[:, CHUNK * b:CHUNK * b + N], ps[:, 0:N],
                            mybir.ActivationFunctionType.Identity,
                            bias=bias_e[:, :], scale=inv[:, :],
                        )
                    else:
                        eng.tensor_scalar(
                            out=st[:, CHUNK * b:CHUNK * b + N], in0=ps[:, 0:N],
                            scalar1=inv[:, :], scalar2=bias_e[:, :],
                            op0=mybir.AluOpType.mult, op1=mybir.AluOpType.add,
                        )
                out_q[g_out % 2].dma_start(
                    out=out_ext[:, j0:j0 + glen], in_=st[:, 0:glen]
                )
                ch += nb
                g_out += 1

    nc.compile()
    return nc


def _get_nc():
    if "nc" not in _cached:
        _cached["nc"] = _build()
    return _cached["nc"]


def make_core_inputs(x, weight, bias):
    """Shard full inputs into per-core input maps (host side)."""
    x = np.ascontiguousarray(x, dtype=np.float32)
    weight = np.ascontiguousarray(weight, dtype=np.float32)
    bias = np.ascontiguousarray(bias, dtype=np.float32)
    in_maps = []
    for core in range(N_CORES):
        b, h = divmod(core, 4)
        xc = np.zeros((XROWS, CI, IMG, W), dtype=np.float32)
        lo = ROWS * h - 1                      # x_core row 0 = full row lo
        src_lo, src_hi = max(lo, 0), min(lo + 98, H)
        xc[src_lo - lo:src_hi - lo, :, :, :] = (
            x[IMG * b:IMG * b + IMG, :, src_lo:src_hi, :].transpose(2, 1, 0, 3)
        )
        in_maps.append({"x": xc, "weight": weight, "bias": bias})
    return in_maps


def assemble_output(results):
    """Gather per-core permuted bf16 streams into the full f32 output."""
    out = np.empty((2 * IMG, CO, H, W), dtype=np.float32)
    for core in range(N_CORES):
        b, h = divmod(core, 4)
        # [NM, C] -> [p, co, img, k, WP]  (m = 8p + co)
        arr = np.asarray(results[core]["out"]).astype(np.float32)
        arr = arr.reshape(P, CO, IMG, NK, WP)
        strip = arr[:, :, :, :, 1:W + 1]       # drop pad cols
        s = strip.transpose(2, 1, 3, 0, 4)     # [img, co, k, p, W]
        dst = out[IMG * b:IMG * b + IMG, :, ROWS * h:ROWS * h + ROWS, :]
        dst[:, :, :84, :] = s[:, :, :6, :, :].reshape(IMG, CO, 84, W)
        dst[:, :, 84:, :] = s[:, :, 6, :12, :]
    return out


def kernel(x, weight, bias):
    from concourse.bass_utils import run_bass_kernel_spmd

    nc = _get_nc()
    in_maps = make_core_inputs(x, weight, bias)
    res = run_bass_kernel_spmd(nc, in_maps, core_ids=list(range(N_CORES)))
    return assemble_output(res.results)
